# revision 1
# baseline (speedup 1.0000x reference)
"""Trainium2 Bass kernel for nn_BaseViewTransform (BEVFusion bev_pool / segment-mean).

Pipeline:
  Host (index plane + sharding, derived from the 5 small input matrices):
    - compute per-point voxel/segment ids exactly as the reference (float32
      geometry, truncation toward zero)
    - sort kept points by segment id; shard = contiguous sorted range per core
      (the "shard the N*D*H*W point dimension" strategy); materialize each
      core's shard as a contiguous bf16 point buffer
    - greedy-pack sorted points into 128-point chunks with <= WIN=8 distinct
      segments; each chunk owns a fixed 8-column slot of a 512-column PSUM bank
  Device (single SPMD program, all heavy compute):
    - streams the point shard contiguously (HWDGE, no gather)
    - one bf16 matmul per chunk: feats[128,80]^T @ onehot[128,8] accumulated
      into the chunk's PSUM slot (start=True/stop=True, disjoint slots)
    - per 64-chunk window: PSUM -> SBUF copy -> DMA out
  Host: sum window partials per segment, divide by counts, scatter into the
  dense [1, 80, 360, 360] BEV grid (empty voxels stay 0 like the reference).
"""

import numpy as np
import ml_dtypes

# ---------------- problem constants (hardcoded per task rules) ----------------
IMAGE_SIZE = (256, 704)
FEATURE_SIZE = (32, 88)
XBOUND = (-54.0, 54.0, 0.3)
YBOUND = (-54.0, 54.0, 0.3)
ZBOUND = (-10.0, 10.0, 20.0)
DBOUND = (1.0, 60.0, 0.5)
C_OUT = 80
NX = (360, 360, 1)
NSEG = NX[2] * NX[0] * NX[1]  # 129600
DX = np.array([XBOUND[2], YBOUND[2], ZBOUND[2]], np.float32)
BX = np.array([XBOUND[0] + XBOUND[2] / 2.0,
               YBOUND[0] + YBOUND[2] / 2.0,
               ZBOUND[0] + ZBOUND[2] / 2.0], np.float32)

NCORES = 8
P = 128          # points per chunk (= matmul contraction dim)
WIN = 8          # max distinct segments per chunk (= one-hot width)
CPW = 64         # chunks per 512-column PSUM window
BANK = CPW * WIN  # 512
GG = 64          # chunks per stream-DMA group (multiple of CPW)


def _frustum():
    iH, iW = IMAGE_SIZE
    fH, fW = FEATURE_SIZE
    ds = np.arange(DBOUND[0], DBOUND[1], DBOUND[2], dtype=np.float32)
    xs = np.linspace(0.0, iW - 1.0, fW, dtype=np.float32)
    ys = np.linspace(0.0, iH - 1.0, fH, dtype=np.float32)
    return np.stack(np.broadcast_arrays(
        xs[None, None, :], ys[None, :, None], ds[:, None, None]), -1
    ).astype(np.float32)  # [D, fH, fW, 3]


def _segments(camera_intrinsics, camera2lidar, img_aug_matrix, lidar_aug_matrix):
    """Replicates reference get_geometry + voxelization in numpy float32.
    Returns (seg[Np] int64, kept[Np] bool)."""
    intr = np.asarray(camera_intrinsics, np.float32)
    c2l = np.asarray(camera2lidar, np.float32)
    img_aug = np.asarray(img_aug_matrix, np.float32)
    lidar_aug = np.asarray(lidar_aug_matrix, np.float32)

    intrins = intr[..., :3, :3]
    post_rots = img_aug[..., :3, :3]
    post_trans = img_aug[..., :3, 3]
    rots = c2l[..., :3, :3]
    trans = c2l[..., :3, 3]
    er = lidar_aug[..., :3, :3]
    et = lidar_aug[..., :3, 3]

    f = _frustum()
    pts = f[None, None] - post_trans[:, :, None, None, None, :]
    ipr = np.linalg.inv(post_rots.astype(np.float64)).astype(np.float32)
    pts = np.einsum('bnij,bndhwj->bndhwi', ipr, pts).astype(np.float32)
    pts = np.concatenate([pts[..., :2] * pts[..., 2:3], pts[..., 2:3]], -1)
    iintr = np.linalg.inv(intrins.astype(np.float64)).astype(np.float32)
    comb = np.einsum('bnij,bnjk->bnik', rots, iintr).astype(np.float32)
    pts = (np.einsum('bnij,bndhwj->bndhwi', comb, pts)
           + trans[:, :, None, None, None, :]).astype(np.float32)
    pts = (np.einsum('bij,bndhwj->bndhwi', er, pts)
           + et[:, None, None, None, None, :]).astype(np.float32)

    Np = pts.size // 3
    geom = ((pts - (BX - DX / 2.0)) / DX).astype(np.int32).reshape(Np, 3)
    kept = ((geom[:, 0] >= 0) & (geom[:, 0] < NX[0])
            & (geom[:, 1] >= 0) & (geom[:, 1] < NX[1])
            & (geom[:, 2] >= 0) & (geom[:, 2] < NX[2]))
    seg = (geom[:, 2].astype(np.int64) * (NX[0] * NX[1])
           + geom[:, 0].astype(np.int64) * NX[1]
           + geom[:, 1].astype(np.int64))
    return seg, kept


def _plan(seg, kept):
    """Sort kept points, shard across cores, greedy-chunk.

    Returns per-core: rows (padded point-row ids), rel (one-hot column per
    point, -1 for padding), first_seg/span per chunk, plus counts for the
    final division.
    """
    kidx = np.nonzero(kept)[0].astype(np.int64)
    segk = seg[kidx]
    order = np.argsort(segk, kind='stable')
    rows_sorted = kidx[order]
    seg_sorted = segk[order]
    counts = np.bincount(seg_sorted, minlength=NSEG)

    nk = len(rows_sorted)
    bounds = [int(round(nk * k / NCORES)) for k in range(NCORES + 1)]

    cores = []
    for k in range(NCORES):
        lo, hi = bounds[k], bounds[k + 1]
        sc = seg_sorted[lo:hi]
        n = hi - lo
        rs = np.flatnonzero(np.r_[True, np.diff(sc) != 0])
        rlen = np.diff(np.r_[rs, n])
        rel = np.empty(n, np.int32)
        chunk_start = []
        chunk_len = []
        cs, fill, d = 0, 0, 0
        for r in range(len(rs)):
            rem = int(rlen[r])
            q = int(rs[r])
            took = 0
            while rem > 0:
                if fill == P or d == WIN:
                    chunk_start.append(cs)
                    chunk_len.append(fill)
                    cs += fill
                    fill, d = 0, 0
                take = min(P - fill, rem)
                rel[q + took:q + took + take] = d
                fill += take
                took += take
                rem -= take
                if rem > 0:
                    chunk_start.append(cs)
                    chunk_len.append(fill)
                    cs += fill
                    fill, d = 0, 0
                else:
                    d += 1
        if fill > 0:
            chunk_start.append(cs)
            chunk_len.append(fill)
        cores.append(dict(lo=lo, hi=hi, rel=rel,
                          chunk_start=np.asarray(chunk_start, np.int64),
                          chunk_len=np.asarray(chunk_len, np.int64)))

    nchunk = max(len(c['chunk_start']) for c in cores)
    nchunk = ((nchunk + CPW - 1) // CPW) * CPW

    rows_all = np.zeros((NCORES, nchunk, P), np.int64)
    rel_all = np.full((NCORES, nchunk, P), -1, np.int32)
    slot_seg = np.zeros((NCORES, nchunk, WIN), np.int64)
    span_all = np.zeros((NCORES, nchunk), np.int32)
    for k, c in enumerate(cores):
        lo = c['lo']
        for t, (s0, ln) in enumerate(zip(c['chunk_start'], c['chunk_len'])):
            sl = slice(lo + s0, lo + s0 + ln)
            rows_all[k, t, :ln] = rows_sorted[sl]
            r = c['rel'][s0:s0 + ln]
            rel_all[k, t, :ln] = r
            # the j-th distinct segment of this chunk (sparse segs are NOT
            # consecutive integers, so record them explicitly)
            slot_seg[k, t, r] = seg_sorted[sl]
            span_all[k, t] = r[-1] + 1
    return dict(nchunk=nchunk, rows=rows_all, rel=rel_all,
                slot_seg=slot_seg, span=span_all, counts=counts)


# ---------------- device program ----------------
_COMPILED = {}


def _build_program(nchunk):
    import concourse.tile as tile
    from concourse import bacc, mybir

    if nchunk in _COMPILED:
        return _COMPILED[nchunk]

    nwin = nchunk // CPW
    dt = mybir.dt.bfloat16
    nc = bacc.Bacc("TRN2", target_bir_lowering=False, debug=False,
                   enable_asserts=False, num_devices=NCORES)
    pts = nc.dram_tensor("pts", [P, nchunk * C_OUT], dt,
                         kind="ExternalInput").ap()
    rel = nc.dram_tensor("rel", [P, nchunk], dt,
                         kind="ExternalInput").ap()
    iota = nc.dram_tensor("iota", [P, WIN], dt, kind="ExternalInput").ap()
    wout = nc.dram_tensor("wout", [nwin, C_OUT, BANK], mybir.dt.float32,
                          kind="ExternalOutput").ap()

    with tile.TileContext(nc) as tc:
        import concourse.bass as bass
        with tc.tile_pool(name="const", bufs=1) as constp, \
             tc.tile_pool(name="feat", bufs=6) as featp, \
             tc.tile_pool(name="oh", bufs=4) as ohp, \
             tc.tile_pool(name="stage", bufs=4) as stagep, \
             tc.tile_pool(name="psum", bufs=6, space="PSUM") as psump:
            rel_t = constp.tile([P, nchunk], dt)
            nc.scalar.dma_start(out=rel_t[:], in_=rel[:])
            iota_t = constp.tile([P, WIN], dt)
            nc.scalar.dma_start(out=iota_t[:], in_=iota[:])

            wpg = max(1, GG // CPW)       # windows per feat-DMA group
            f_t = None
            for w in range(nwin):
                ps = psump.tile([P, BANK], mybir.dt.float32)
                # one-hot for the whole window via DVE compare
                oh_w = ohp.tile([P, CPW, WIN], dt)
                rsl = rel_t[:, w * CPW:(w + 1) * CPW]
                rel_b = bass.AP(rsl.tensor, rsl.offset,
                                list(rsl.ap) + [[0, WIN]])
                iap = iota_t[:]
                iota_b = bass.AP(iap.tensor, iap.offset,
                                 [iap.ap[0], [0, CPW], iap.ap[1]])
                nc.vector.tensor_tensor(out=oh_w[:], in0=iota_b, in1=rel_b,
                                        op=mybir.AluOpType.is_equal)
                if w % wpg == 0:
                    t0 = w * CPW
                    ng = min(GG, nchunk - t0)
                    f_t = featp.tile([P, GG, C_OUT], dt)
                    eng = nc.sync if (w // wpg) % 2 == 0 else nc.scalar
                    eng.dma_start(
                        out=f_t[:, :ng],
                        in_=pts[:, t0 * C_OUT:(t0 + ng) * C_OUT].rearrange(
                            "p (t d) -> p t d", d=C_OUT))
                for c in range(CPW):
                    col = c * WIN
                    nc.tensor.matmul(
                        out=ps[:C_OUT, col:col + WIN],
                        lhsT=f_t[:, (w % wpg) * CPW + c],
                        rhs=oh_w[:, c],
                        start=True,
                        stop=True,
                    )
                st = stagep.tile([C_OUT, BANK], mybir.dt.float32)
                nc.vector.tensor_copy(out=st[:], in_=ps[:C_OUT])
                nc.scalar.dma_start(out=wout[w], in_=st[:])

    nc.compile()
    _COMPILED[nchunk] = nc
    return nc


def _run_on_hw(nc, in_maps, trace=False):
    from concourse.bass_utils import run_bass_kernel_spmd
    from concourse.bass_interp import get_hw_module

    if trace:
        try:
            import ntff_hook
            ntff_hook.install()
        except Exception:
            pass
    hw_m = get_hw_module(nc.m)
    old_m = nc.m
    nc.m = hw_m
    try:
        res = run_bass_kernel_spmd(
            nc, in_maps, core_ids=list(range(NCORES)), trace=trace,
        )
    finally:
        nc.m = old_m
    return res


def kernel(cam_feats, camera_intrinsics, camera2lidar, img_aug_matrix,
           lidar_aug_matrix, _trace=False, _return_results=False):
    cam = np.ascontiguousarray(np.asarray(cam_feats, np.float32))
    Npts = cam.size // C_OUT
    cam_bf = cam.reshape(Npts, C_OUT).astype(ml_dtypes.bfloat16)

    seg, kept = _segments(camera_intrinsics, camera2lidar,
                          img_aug_matrix, lidar_aug_matrix)
    plan = _plan(seg, kept)
    nchunk = plan['nchunk']

    # per-core contiguous shard: [P, nchunk, C_OUT] (partition-major stream)
    iota_c = np.broadcast_to(np.arange(WIN, dtype=np.float32),
                             (P, WIN)).astype(ml_dtypes.bfloat16)
    in_maps = []
    for k in range(NCORES):
        shard = cam_bf[plan['rows'][k].reshape(-1)]
        shard = shard.reshape(nchunk, P, C_OUT).transpose(1, 0, 2)
        shard = np.ascontiguousarray(shard).reshape(P, nchunk * C_OUT)
        # padding points (rel == -1) match no iota column -> zero one-hot row
        relk = np.ascontiguousarray(
            plan['rel'][k].T.astype(np.float32)).astype(ml_dtypes.bfloat16)
        in_maps.append(dict(pts=shard, rel=relk, iota=iota_c))

    nc = _build_program(nchunk)
    res = _run_on_hw(nc, in_maps, trace=_trace)

    # ---------------- host assembly ----------------
    nwin = nchunk // CPW
    vals = np.stack([np.asarray(r['wout']).astype(np.float32)
                     for r in res.results])
    vals = vals.reshape(NCORES, nwin, C_OUT, CPW, WIN)
    vals = vals.transpose(0, 1, 3, 4, 2).reshape(NCORES, nchunk * WIN, C_OUT)

    segs = plan['slot_seg']
    valid = (np.arange(WIN)[None, None, :] < plan['span'][:, :, None])
    s_all = segs.reshape(NCORES, nchunk * WIN)[valid.reshape(NCORES, -1)]
    v_all = vals[valid.reshape(NCORES, -1)]
    o2 = np.argsort(s_all, kind='stable')
    s2 = s_all[o2]
    v2 = v_all[o2]
    acc = np.zeros((NSEG, C_OUT), np.float32)
    if len(s2):
        starts = np.r_[0, np.flatnonzero(np.diff(s2)) + 1]
        sums = np.add.reduceat(v2, starts, axis=0)
        useg = s2[starts]
        acc[useg] = sums / np.maximum(plan['counts'][useg], 1)[:, None]

    out = acc.reshape(NX[2], NX[0], NX[1], C_OUT).transpose(0, 3, 1, 2)
    out = out.reshape(1, NX[2] * C_OUT, NX[0], NX[1]).astype(np.float32)
    if _return_results:
        return out, res
    return out



# revision 2
# speedup vs baseline: 1.0694x; 1.0694x over previous
"""Trainium2 Bass kernel for nn_BaseViewTransform (BEVFusion bev_pool / segment-mean).

Pipeline (v2 — int8 stream + device pair-add + swapped matmul operands):
  Host (index plane only, derived from the 5 small input matrices):
    - compute per-point voxel/segment ids exactly as the reference
    - sort kept points by segment id; pair adjacent same-segment points into
      pseudo-points (odd tails padded with a zero slot)
    - shard pseudo-points across 8 cores; pack into 128-pseudo chunks and
      SC-chunk windows with <= WIN distinct segments per window
    - per-window int8 quantization (symmetric, scale = absmax/127)
  Device (single SPMD program):
    - stream int8 point pairs contiguously (HWDGE)
    - DVE/GPSIMD: pair-add int8+int8 -> bf16 (exact: |q1+q2| <= 254)
    - one bf16 matmul per chunk with the ONE-HOT as the stationary operand
      (LDWEIGHTS cost ~ WIN columns, not 80) streaming the 80 feature
      channels; SC chunks accumulate into a shared PSUM [WIN, 80] slot
    - ACT: PSUM -> SBUF bf16 copy per 6-window bank group; DMA out
  Host: scale window sums by the window scale, reduce per segment across
  windows/cores, divide by counts, scatter into the [1, 80, 360, 360] grid.
"""

import numpy as np
import ml_dtypes

# ---------------- problem constants (hardcoded per task rules) ----------------
IMAGE_SIZE = (256, 704)
FEATURE_SIZE = (32, 88)
XBOUND = (-54.0, 54.0, 0.3)
YBOUND = (-54.0, 54.0, 0.3)
ZBOUND = (-10.0, 10.0, 20.0)
DBOUND = (1.0, 60.0, 0.5)
C_OUT = 80
NX = (360, 360, 1)
NSEG = NX[2] * NX[0] * NX[1]  # 129600
DX = np.array([XBOUND[2], YBOUND[2], ZBOUND[2]], np.float32)
BX = np.array([XBOUND[0] + XBOUND[2] / 2.0,
               YBOUND[0] + YBOUND[2] / 2.0,
               ZBOUND[0] + ZBOUND[2] / 2.0], np.float32)

NCORES = 8
P = 128          # pseudo-points per chunk (= matmul contraction dim)
G = 2            # original points per pseudo-point (device pair-add)
SC = 2           # chunks accumulated per PSUM window
WIN = 32         # max distinct segments per window (= PSUM slot partitions)
SPB = 6          # windows per PSUM bank group (6*80 = 480 <= 512 fp32)
GCH = SC * SPB   # chunks per bank group
FW = G * C_OUT   # feature elements per chunk per partition
DVE_SHARE = 3    # of every 5 groups, this many pair-adds go to DVE (rest gpsimd)


def _frustum():
    iH, iW = IMAGE_SIZE
    fH, fW = FEATURE_SIZE
    ds = np.arange(DBOUND[0], DBOUND[1], DBOUND[2], dtype=np.float32)
    xs = np.linspace(0.0, iW - 1.0, fW, dtype=np.float32)
    ys = np.linspace(0.0, iH - 1.0, fH, dtype=np.float32)
    return np.stack(np.broadcast_arrays(
        xs[None, None, :], ys[None, :, None], ds[:, None, None]), -1
    ).astype(np.float32)  # [D, fH, fW, 3]


def _segments(camera_intrinsics, camera2lidar, img_aug_matrix, lidar_aug_matrix):
    """Replicates reference get_geometry + voxelization in numpy float32.
    Returns (seg[Np] int64, kept[Np] bool)."""
    intr = np.asarray(camera_intrinsics, np.float32)
    c2l = np.asarray(camera2lidar, np.float32)
    img_aug = np.asarray(img_aug_matrix, np.float32)
    lidar_aug = np.asarray(lidar_aug_matrix, np.float32)

    intrins = intr[..., :3, :3]
    post_rots = img_aug[..., :3, :3]
    post_trans = img_aug[..., :3, 3]
    rots = c2l[..., :3, :3]
    trans = c2l[..., :3, 3]
    er = lidar_aug[..., :3, :3]
    et = lidar_aug[..., :3, 3]

    f = _frustum()
    pts = f[None, None] - post_trans[:, :, None, None, None, :]
    ipr = np.linalg.inv(post_rots.astype(np.float64)).astype(np.float32)
    pts = np.einsum('bnij,bndhwj->bndhwi', ipr, pts).astype(np.float32)
    pts = np.concatenate([pts[..., :2] * pts[..., 2:3], pts[..., 2:3]], -1)
    iintr = np.linalg.inv(intrins.astype(np.float64)).astype(np.float32)
    comb = np.einsum('bnij,bnjk->bnik', rots, iintr).astype(np.float32)
    pts = (np.einsum('bnij,bndhwj->bndhwi', comb, pts)
           + trans[:, :, None, None, None, :]).astype(np.float32)
    pts = (np.einsum('bij,bndhwj->bndhwi', er, pts)
           + et[:, None, None, None, None, :]).astype(np.float32)

    Np_ = pts.size // 3
    geom = ((pts - (BX - DX / 2.0)) / DX).astype(np.int32).reshape(Np_, 3)
    kept = ((geom[:, 0] >= 0) & (geom[:, 0] < NX[0])
            & (geom[:, 1] >= 0) & (geom[:, 1] < NX[1])
            & (geom[:, 2] >= 0) & (geom[:, 2] < NX[2]))
    seg = (geom[:, 2].astype(np.int64) * (NX[0] * NX[1])
           + geom[:, 0].astype(np.int64) * NX[1]
           + geom[:, 1].astype(np.int64))
    return seg, kept


def _plan(seg, kept):
    """Sort kept points, pair into pseudo-points, shard, pack into windows.

    Returns per-core rows [nchunk,P,G] (point row ids, -1 pad), rel [nchunk,P]
    (slot in window, -1 pad), win_seg [nwin,WIN] (segment per slot, -1 unused),
    plus global counts.
    """
    kidx = np.nonzero(kept)[0].astype(np.int64)
    segk = seg[kidx]
    order = np.argsort(segk, kind='stable')
    rows_sorted = kidx[order]
    seg_sorted = segk[order]
    counts = np.bincount(seg_sorted, minlength=NSEG).astype(np.float32)

    rs = np.flatnonzero(np.r_[True, np.diff(seg_sorted) != 0]).astype(np.int64)
    rlen = np.diff(np.r_[rs, len(seg_sorted)]).astype(np.int64)
    run_seg = seg_sorted[rs]
    plen = (rlen + G - 1) // G
    pcum = np.concatenate([[0], np.cumsum(plen)])
    npseudo = int(pcum[-1])
    bounds = [int(round(npseudo * k / NCORES)) for k in range(NCORES + 1)]
    WCAP = SC * P

    cores = []
    ri = 0
    for k in range(NCORES):
        lo, hi = bounds[k], bounds[k + 1]
        while ri + 1 < len(pcum) and pcum[ri + 1] <= lo:
            ri += 1
        # pieces: (run_id, take, run_poff, slot, window); run_id=-1 -> dead
        pieces = []
        q = 0
        d = 0
        p = lo
        rj = ri
        while p < hi:
            run_end = pcum[rj + 1]
            rem = int(min(run_end, hi) - p)
            poff = int(p - pcum[rj])
            while rem:
                wpos = q % WCAP
                if wpos == 0:
                    d = 0
                room = WCAP - wpos
                if d == WIN:
                    pieces.append((-1, room, 0, 0, q // WCAP))
                    q += room
                    continue
                slot = d
                d += 1
                take = min(rem, room)
                pieces.append((rj, take, poff, slot, q // WCAP))
                q += take
                rem -= take
                poff += take
            p = int(min(run_end, hi))
            if p >= run_end:
                rj += 1
        cores.append((pieces, q))

    nchunk = max((q + P - 1) // P for _, q in cores)
    nchunk = ((nchunk + GCH - 1) // GCH) * GCH
    nwin = nchunk // SC

    out = []
    for k, (pieces, q) in enumerate(cores):
        Q = nchunk * P
        run_f = np.full(Q, -1, np.int64)
        poff_f = np.zeros(Q, np.int64)
        rel_f = np.full(Q, -1, np.int32)
        win_seg = np.full((nwin, WIN), -1, np.int64)
        pos = 0
        for (rj, take, poff, slot, w) in pieces:
            if rj >= 0:
                run_f[pos:pos + take] = rj
                poff_f[pos:pos + take] = np.arange(poff, poff + take)
                rel_f[pos:pos + take] = slot
                win_seg[w, slot] = run_seg[rj]
            pos += take
        valid = run_f >= 0
        rows = np.full((Q, G), -1, np.int64)
        base = rs[run_f[valid]] + poff_f[valid] * G
        end = rs[run_f[valid]] + rlen[run_f[valid]]
        for j in range(G):
            idx = base + j
            ok = idx < end
            rr = np.full(valid.sum(), -1, np.int64)
            rr[ok] = rows_sorted[idx[ok]]
            rows[valid, j] = rr
        out.append(dict(rows=rows.reshape(nchunk, P, G),
                        rel=rel_f.reshape(nchunk, P).astype(np.int32),
                        win_seg=win_seg))
    return dict(nchunk=nchunk, nwin=nwin, cores=out, counts=counts)


# ---------------- device program ----------------
_COMPILED = {}


def _build_program(nchunk):
    import concourse.tile as tile
    from concourse import bacc, mybir
    import concourse.bass as bass

    key = (nchunk, G)
    if key in _COMPILED:
        return _COMPILED[key]

    ngroup = nchunk // GCH
    bf = mybir.dt.bfloat16
    ftype = mybir.dt.int8 if G == 2 else bf
    nc = bacc.Bacc("TRN2", target_bir_lowering=False, debug=False,
                   enable_asserts=False, num_devices=NCORES)
    pts = nc.dram_tensor("pts", [P, nchunk * FW], ftype,
                         kind="ExternalInput").ap()
    rel = nc.dram_tensor("rel", [P, nchunk], bf, kind="ExternalInput").ap()
    iota = nc.dram_tensor("iota", [P, WIN], bf, kind="ExternalInput").ap()
    wout = nc.dram_tensor("wout", [ngroup, WIN, SPB * C_OUT], bf,
                          kind="ExternalOutput").ap()

    with tile.TileContext(nc) as tc:
        with tc.tile_pool(name="const", bufs=1) as constp, \
             tc.tile_pool(name="feat", bufs=4) as featp, \
             tc.tile_pool(name="pair", bufs=4) as pairp, \
             tc.tile_pool(name="oh", bufs=3) as ohp, \
             tc.tile_pool(name="stage", bufs=4) as stagep, \
             tc.tile_pool(name="psum", bufs=4, space="PSUM") as psump:
            rel_t = constp.tile([P, nchunk], bf)
            nc.sync.dma_start(out=rel_t[:], in_=rel[:])
            iota_t = constp.tile([P, WIN], bf)
            nc.sync.dma_start(out=iota_t[:], in_=iota[:])

            for g in range(ngroup):
                c0 = g * GCH
                f_t = featp.tile([P, GCH * FW], ftype)
                eng = nc.sync if g % 2 == 0 else nc.scalar
                eng.dma_start(out=f_t[:], in_=pts[:, c0 * FW:(c0 + GCH) * FW])

                # one-hot for the whole group via DVE compare (bf16, 2x mode)
                oh = ohp.tile([P, GCH, WIN], bf)
                rsl = rel_t[:, c0:c0 + GCH]
                rel_b = bass.AP(rsl.tensor, rsl.offset,
                                list(rsl.ap) + [[0, WIN]])
                iap = iota_t[:]
                iota_b = bass.AP(iap.tensor, iap.offset,
                                 [iap.ap[0], [0, GCH], iap.ap[1]])
                nc.vector.tensor_tensor(out=oh[:], in0=iota_b, in1=rel_b,
                                        op=mybir.AluOpType.is_equal)

                if G == 2:
                    # pair-add int8+int8 -> bf16 for the whole group
                    pr = pairp.tile([P, GCH, C_OUT], bf)
                    fsl = f_t[:]
                    in0 = bass.AP(fsl.tensor, fsl.offset,
                                  [fsl.ap[0], [FW, GCH], [1, C_OUT]])
                    in1 = bass.AP(fsl.tensor, fsl.offset + C_OUT,
                                  [fsl.ap[0], [FW, GCH], [1, C_OUT]])
                    peng = nc.vector if (g % 5) < DVE_SHARE else nc.gpsimd
                    peng.tensor_tensor(out=pr[:], in0=in0, in1=in1,
                                       op=mybir.AluOpType.add)

                ps = psump.tile([P, 512], mybir.dt.float32)
                for c in range(GCH):
                    s = c // SC
                    if G == 2:
                        rhs = pr[:, c]
                    else:
                        rhs = f_t[:, c * FW:(c + 1) * FW]
                    nc.tensor.matmul(
                        out=ps[0:WIN, s * C_OUT:(s + 1) * C_OUT],
                        lhsT=oh[:, c],
                        rhs=rhs,
                        start=(c % SC == 0),
                        stop=(c % SC == SC - 1),
                    )
                st = stagep.tile([WIN, SPB * C_OUT], bf)
                nc.scalar.copy(out=st[:], in_=ps[0:WIN, 0:SPB * C_OUT])
                nc.gpsimd.dma_start(out=wout[g], in_=st[:])

    nc.compile()
    _COMPILED[key] = nc
    return nc


def _run_on_hw(nc, in_maps, trace=False):
    from concourse.bass_utils import run_bass_kernel_spmd
    from concourse.bass_interp import get_hw_module

    if trace:
        try:
            import ntff_hook
            ntff_hook.install()
        except Exception:
            pass
    hw_m = get_hw_module(nc.m)
    old_m = nc.m
    nc.m = hw_m
    try:
        res = run_bass_kernel_spmd(
            nc, in_maps, core_ids=list(range(NCORES)), trace=trace,
        )
    finally:
        nc.m = old_m
    return res


def kernel(cam_feats, camera_intrinsics, camera2lidar, img_aug_matrix,
           lidar_aug_matrix, _trace=False, _return_results=False):
    cam = np.ascontiguousarray(np.asarray(cam_feats, np.float32))
    Npts = cam.size // C_OUT
    cam_f = cam.reshape(Npts, C_OUT)
    cam_aug = np.vstack([cam_f, np.zeros((1, C_OUT), np.float32)])

    seg, kept = _segments(camera_intrinsics, camera2lidar,
                          img_aug_matrix, lidar_aug_matrix)
    plan = _plan(seg, kept)
    nchunk = plan['nchunk']
    nwin = plan['nwin']

    iota_c = np.broadcast_to(np.arange(WIN, dtype=np.float32),
                             (P, WIN)).astype(ml_dtypes.bfloat16)
    in_maps = []
    scales = []
    for k in range(NCORES):
        ck = plan['cores'][k]
        f = cam_aug[ck['rows'].reshape(-1)].reshape(nchunk, P, G, C_OUT)
        if G == 2:
            a = np.abs(f).reshape(nwin, SC * P * G * C_OUT).max(axis=1)
            s = np.where(a > 0, a / 127.0, 1.0).astype(np.float32)
            q = np.rint(f / s.repeat(SC)[:, None, None, None])
            shard = np.clip(q, -127, 127).astype(np.int8)
        else:
            s = np.ones(nwin, np.float32)
            shard = f.astype(ml_dtypes.bfloat16)
        scales.append(s)
        shard = np.ascontiguousarray(shard.transpose(1, 0, 2, 3)
                                     ).reshape(P, nchunk * FW)
        relk = np.ascontiguousarray(
            ck['rel'].T.astype(np.float32)).astype(ml_dtypes.bfloat16)
        in_maps.append(dict(pts=shard, rel=relk, iota=iota_c))

    nc = _build_program(nchunk)
    res = _run_on_hw(nc, in_maps, trace=_trace)

    # ---------------- host assembly ----------------
    ngroup = nchunk // GCH
    s_parts = []
    v_parts = []
    for k in range(NCORES):
        vals = np.asarray(res.results[k]['wout']).astype(np.float32)
        vals = vals.reshape(ngroup, WIN, SPB, C_OUT).transpose(0, 2, 1, 3)
        vals = vals.reshape(nwin, WIN, C_OUT) * scales[k][:, None, None]
        wseg = plan['cores'][k]['win_seg']
        m = wseg >= 0
        s_parts.append(wseg[m])
        v_parts.append(vals[m])
    s_all = np.concatenate(s_parts)
    v_all = np.concatenate(v_parts)
    acc = np.zeros((NSEG, C_OUT), np.float32)
    if len(s_all):
        o2 = np.argsort(s_all, kind='stable')
        s2 = s_all[o2]
        v2 = v_all[o2]
        starts = np.r_[0, np.flatnonzero(np.diff(s2)) + 1]
        sums = np.add.reduceat(v2, starts, axis=0)
        useg = s2[starts]
        acc[useg] = sums / np.maximum(plan['counts'][useg], 1.0)[:, None]

    out = acc.reshape(NX[2], NX[0], NX[1], C_OUT).transpose(0, 3, 1, 2)
    out = out.reshape(1, NX[2] * C_OUT, NX[0], NX[1]).astype(np.float32)
    if _return_results:
        return out, res
    return out


# revision 3
# speedup vs baseline: 1.2377x; 1.1574x over previous
"""Trainium2 Bass kernel for nn_BaseViewTransform (BEVFusion bev_pool / segment-mean).

Pipeline (v2b — int8 stream, device pair-add, feats-stationary FWL matmul):
  Host (index plane only, derived from the 5 small input matrices):
    - compute per-point voxel/segment ids exactly as the reference
    - sort kept points by segment id; pair adjacent same-segment points into
      pseudo-points (odd tails padded with a zero slot)
    - shard pseudo-points across 8 cores; greedy-pack into 128-pseudo chunks
      with <= WIN=12 distinct segments per chunk
    - per-chunk symmetric int8 quantization (scale = absmax/127)
  Device (single SPMD program), per 42-chunk window:
    - stream int8 points (evens block + odds block, contiguous)
    - DVE + GPSIMD split: pair-add int8+int8 -> bf16 (exact: |q1+q2| <= 254)
    - DVE: one-hot [128, 42, 12] via is_equal on iota/rel
    - 42 matmuls: pair-sums stationary (lhsT padded to 128 cols so FWL
      triggers), one-hot moving (12 cols); PSUM [128, 42*12]
    - ACT: PSUM [0:80] -> SBUF bf16 copy; DMA out
  Host: scale chunk sums by the chunk scale, reduce per segment, divide by
  counts, scatter into the [1, 80, 360, 360] grid.
"""

import numpy as np
import ml_dtypes

# ---------------- problem constants (hardcoded per task rules) ----------------
IMAGE_SIZE = (256, 704)
FEATURE_SIZE = (32, 88)
XBOUND = (-54.0, 54.0, 0.3)
YBOUND = (-54.0, 54.0, 0.3)
ZBOUND = (-10.0, 10.0, 20.0)
DBOUND = (1.0, 60.0, 0.5)
C_OUT = 80
NX = (360, 360, 1)
NSEG = NX[2] * NX[0] * NX[1]  # 129600
DX = np.array([XBOUND[2], YBOUND[2], ZBOUND[2]], np.float32)
BX = np.array([XBOUND[0] + XBOUND[2] / 2.0,
               YBOUND[0] + YBOUND[2] / 2.0,
               ZBOUND[0] + ZBOUND[2] / 2.0], np.float32)

NCORES = 8
P = 128          # pseudo-points per chunk (= matmul contraction dim)
G = 2            # original points per pseudo-point (device pair-add)
WIN = 12         # max distinct segments per chunk (= one-hot width)
CPW = 42         # chunks per window (42*12 = 504 <= 512 fp32 PSUM bank)
FW = G * C_OUT   # feature elements per chunk per partition (160)
NS = 21          # chunks of each window pair-added on DVE (rest on GPSIMD)
PAD = P - C_OUT  # lhsT column padding to reach 128 (junk-read)


def _frustum():
    iH, iW = IMAGE_SIZE
    fH, fW = FEATURE_SIZE
    ds = np.arange(DBOUND[0], DBOUND[1], DBOUND[2], dtype=np.float32)
    xs = np.linspace(0.0, iW - 1.0, fW, dtype=np.float32)
    ys = np.linspace(0.0, iH - 1.0, fH, dtype=np.float32)
    return np.stack(np.broadcast_arrays(
        xs[None, None, :], ys[None, :, None], ds[:, None, None]), -1
    ).astype(np.float32)  # [D, fH, fW, 3]


def _segments(camera_intrinsics, camera2lidar, img_aug_matrix, lidar_aug_matrix):
    """Replicates reference get_geometry + voxelization in numpy float32.
    Returns (seg[Np] int64, kept[Np] bool)."""
    intr = np.asarray(camera_intrinsics, np.float32)
    c2l = np.asarray(camera2lidar, np.float32)
    img_aug = np.asarray(img_aug_matrix, np.float32)
    lidar_aug = np.asarray(lidar_aug_matrix, np.float32)

    intrins = intr[..., :3, :3]
    post_rots = img_aug[..., :3, :3]
    post_trans = img_aug[..., :3, 3]
    rots = c2l[..., :3, :3]
    trans = c2l[..., :3, 3]
    er = lidar_aug[..., :3, :3]
    et = lidar_aug[..., :3, 3]

    f = _frustum()
    pts = f[None, None] - post_trans[:, :, None, None, None, :]
    ipr = np.linalg.inv(post_rots.astype(np.float64)).astype(np.float32)
    pts = np.einsum('bnij,bndhwj->bndhwi', ipr, pts).astype(np.float32)
    pts = np.concatenate([pts[..., :2] * pts[..., 2:3], pts[..., 2:3]], -1)
    iintr = np.linalg.inv(intrins.astype(np.float64)).astype(np.float32)
    comb = np.einsum('bnij,bnjk->bnik', rots, iintr).astype(np.float32)
    pts = (np.einsum('bnij,bndhwj->bndhwi', comb, pts)
           + trans[:, :, None, None, None, :]).astype(np.float32)
    pts = (np.einsum('bij,bndhwj->bndhwi', er, pts)
           + et[:, None, None, None, None, :]).astype(np.float32)

    Np_ = pts.size // 3
    geom = ((pts - (BX - DX / 2.0)) / DX).astype(np.int32).reshape(Np_, 3)
    kept = ((geom[:, 0] >= 0) & (geom[:, 0] < NX[0])
            & (geom[:, 1] >= 0) & (geom[:, 1] < NX[1])
            & (geom[:, 2] >= 0) & (geom[:, 2] < NX[2]))
    seg = (geom[:, 2].astype(np.int64) * (NX[0] * NX[1])
           + geom[:, 0].astype(np.int64) * NX[1]
           + geom[:, 1].astype(np.int64))
    return seg, kept


def _plan(seg, kept):
    """Sort kept points, pair into pseudo-points, shard, greedy-chunk.

    Returns per-core rows [nchunk,P,G] (point row ids, -1 pad), rel [nchunk,P]
    (one-hot column, -1 pad), slot_seg [nchunk,WIN] (segment per slot, -1
    unused), plus global counts.
    """
    kidx = np.nonzero(kept)[0].astype(np.int64)
    segk = seg[kidx]
    order = np.argsort(segk, kind='stable')
    rows_sorted = kidx[order]
    seg_sorted = segk[order]
    counts = np.bincount(seg_sorted, minlength=NSEG).astype(np.float32)

    rs = np.flatnonzero(np.r_[True, np.diff(seg_sorted) != 0]).astype(np.int64)
    rlen = np.diff(np.r_[rs, len(seg_sorted)]).astype(np.int64)
    run_seg = seg_sorted[rs]
    plen = (rlen + G - 1) // G
    pcum = np.concatenate([[0], np.cumsum(plen)])
    npseudo = int(pcum[-1])
    bounds = [int(round(npseudo * k / NCORES)) for k in range(NCORES + 1)]

    cores_pieces = []
    ri = 0
    for k in range(NCORES):
        lo, hi = bounds[k], bounds[k + 1]
        while ri + 1 < len(pcum) and pcum[ri + 1] <= lo:
            ri += 1
        # pieces: (run_id, take, run_poff, slot, chunk)
        pieces = []
        q = 0   # local pseudo cursor incl. padding
        d = 0   # distinct segs in current chunk
        p = lo
        rj = ri
        while p < hi:
            run_end = pcum[rj + 1]
            rem = int(min(run_end, hi) - p)
            poff = int(p - pcum[rj])
            while rem:
                cpos = q % P
                if cpos == 0:
                    d = 0
                if d == WIN:  # chunk out of slots: pad to chunk end
                    q += P - cpos
                    d = 0
                    cpos = 0
                slot = d
                d += 1
                take = min(rem, P - cpos)
                pieces.append((rj, take, poff, slot, q // P))
                q += take
                rem -= take
                poff += take
            p = int(min(run_end, hi))
            if p >= run_end:
                rj += 1
        cores_pieces.append((pieces, q))

    nchunk = max((q + P - 1) // P for _, q in cores_pieces)
    nchunk = ((nchunk + CPW - 1) // CPW) * CPW

    out = []
    for k, (pieces, q) in enumerate(cores_pieces):
        Q = nchunk * P
        run_f = np.full(Q, -1, np.int64)
        poff_f = np.zeros(Q, np.int64)
        rel_f = np.full(Q, -1, np.int32)
        slot_seg = np.full((nchunk, WIN), -1, np.int64)
        pos = 0
        for (rj, take, poff, slot, c) in pieces:
            cpos = pos % P
            if pos // P != c:  # padding was inserted before this piece
                pos = c * P
            run_f[pos:pos + take] = rj
            poff_f[pos:pos + take] = np.arange(poff, poff + take)
            rel_f[pos:pos + take] = slot
            slot_seg[c, slot] = run_seg[rj]
            pos += take
        valid = run_f >= 0
        rows = np.full((Q, G), -1, np.int64)
        base = rs[run_f[valid]] + poff_f[valid] * G
        end = rs[run_f[valid]] + rlen[run_f[valid]]
        for j in range(G):
            idx = base + j
            ok = idx < end
            rr = np.full(int(valid.sum()), -1, np.int64)
            rr[ok] = rows_sorted[idx[ok]]
            rows[valid, j] = rr
        out.append(dict(rows=rows.reshape(nchunk, P, G),
                        rel=rel_f.reshape(nchunk, P).astype(np.int32),
                        slot_seg=slot_seg))
    return dict(nchunk=nchunk, cores=out, counts=counts)


# ---------------- device program ----------------
_COMPILED = {}


def _build_program(nchunk):
    import concourse.tile as tile
    from concourse import bacc, mybir
    import concourse.bass as bass

    key = (nchunk, G, WIN)
    if key in _COMPILED:
        return _COMPILED[key]

    nwin = nchunk // CPW
    bf = mybir.dt.bfloat16
    i8 = mybir.dt.int8
    WB = CPW * C_OUT            # feature elems per window block (3360)
    nc = bacc.Bacc("TRN2", target_bir_lowering=False, debug=False,
                   enable_asserts=False, num_devices=NCORES)
    pts = nc.dram_tensor("pts", [P, nwin * 2 * WB], i8,
                         kind="ExternalInput").ap()
    rel = nc.dram_tensor("rel", [P, nchunk], bf, kind="ExternalInput").ap()
    iota = nc.dram_tensor("iota", [P, WIN], bf, kind="ExternalInput").ap()
    wout = nc.dram_tensor("wout", [nwin, C_OUT, CPW * WIN], bf,
                          kind="ExternalOutput").ap()

    with tile.TileContext(nc) as tc:
        with tc.tile_pool(name="const", bufs=1) as constp, \
             tc.tile_pool(name="feat", bufs=3) as featp, \
             tc.tile_pool(name="pair", bufs=3) as pairp, \
             tc.tile_pool(name="oh", bufs=3) as ohp, \
             tc.tile_pool(name="stage", bufs=3) as stagep, \
             tc.tile_pool(name="psum", bufs=4, space="PSUM") as psump:
            rel_t = constp.tile([P, nchunk], bf)
            nc.sync.dma_start(out=rel_t[:], in_=rel[:])
            iota_t = constp.tile([P, WIN], bf)
            nc.sync.dma_start(out=iota_t[:], in_=iota[:])

            for w in range(nwin):
                f_t = featp.tile([P, 2 * WB], i8)
                eng = nc.sync if w % 2 == 0 else nc.scalar
                eng.dma_start(out=f_t[:], in_=pts[:, w * 2 * WB:(w + 1) * 2 * WB])

                # pair-add int8+int8 -> bf16, split DVE / GPSIMD
                pr = pairp.tile([P, WB + PAD], bf)
                sp = NS * C_OUT
                nc.vector.tensor_tensor(
                    out=pr[:, 0:sp], in0=f_t[:, 0:sp],
                    in1=f_t[:, WB:WB + sp], op=mybir.AluOpType.add)
                nc.gpsimd.tensor_tensor(
                    out=pr[:, sp:WB], in0=f_t[:, sp:WB],
                    in1=f_t[:, WB + sp:2 * WB], op=mybir.AluOpType.add)

                # one-hot for the window via DVE compare
                oh = ohp.tile([P, CPW, WIN], bf)
                rsl = rel_t[:, w * CPW:(w + 1) * CPW]
                rel_b = bass.AP(rsl.tensor, rsl.offset,
                                list(rsl.ap) + [[0, WIN]])
                iap = iota_t[:]
                iota_b = bass.AP(iap.tensor, iap.offset,
                                 [iap.ap[0], [0, CPW], iap.ap[1]])
                nc.vector.tensor_tensor(out=oh[:], in0=iota_b, in1=rel_b,
                                        op=mybir.AluOpType.is_equal)

                ps = psump.tile([P, 512], mybir.dt.float32)
                for c in range(CPW):
                    # stationary = pair sums, padded to 128 cols (junk-read)
                    lhsT = pr[:, c * C_OUT:c * C_OUT + P]
                    nc.tensor.matmul(
                        out=ps[0:P, c * WIN:(c + 1) * WIN],
                        lhsT=lhsT,
                        rhs=oh[:, c],
                        start=True,
                        stop=True,
                    )
                st = stagep.tile([C_OUT, CPW * WIN], bf)
                nc.scalar.copy(out=st[:], in_=ps[0:C_OUT, 0:CPW * WIN])
                nc.gpsimd.dma_start(out=wout[w], in_=st[:])

    nc.compile()
    _COMPILED[key] = nc
    return nc


def _run_on_hw(nc, in_maps, trace=False):
    from concourse.bass_utils import run_bass_kernel_spmd
    from concourse.bass_interp import get_hw_module

    if trace:
        try:
            import ntff_hook
            ntff_hook.install()
        except Exception:
            pass
    hw_m = get_hw_module(nc.m)
    old_m = nc.m
    nc.m = hw_m
    try:
        res = run_bass_kernel_spmd(
            nc, in_maps, core_ids=list(range(NCORES)), trace=trace,
        )
    finally:
        nc.m = old_m
    return res


def kernel(cam_feats, camera_intrinsics, camera2lidar, img_aug_matrix,
           lidar_aug_matrix, _trace=False, _return_results=False):
    cam = np.ascontiguousarray(np.asarray(cam_feats, np.float32))
    Npts = cam.size // C_OUT
    cam_f = cam.reshape(Npts, C_OUT)
    cam_aug = np.vstack([cam_f, np.zeros((1, C_OUT), np.float32)])

    seg, kept = _segments(camera_intrinsics, camera2lidar,
                          img_aug_matrix, lidar_aug_matrix)
    plan = _plan(seg, kept)
    nchunk = plan['nchunk']
    nwin = nchunk // CPW

    iota_c = np.broadcast_to(np.arange(WIN, dtype=np.float32),
                             (P, WIN)).astype(ml_dtypes.bfloat16)
    in_maps = []
    scales = []
    for k in range(NCORES):
        ck = plan['cores'][k]
        f = cam_aug[ck['rows'].reshape(-1)].reshape(nchunk, P, G, C_OUT)
        a = np.abs(f).reshape(nchunk, P * G * C_OUT).max(axis=1)
        s = np.where(a > 0, a / 127.0, 1.0).astype(np.float32)
        q = np.rint(f / s[:, None, None, None])
        q = np.clip(q, -127, 127).astype(np.int8)
        scales.append(s)
        # [nchunk,P,G,C] -> [P, nwin, G, CPW, C] (evens block, odds block)
        shard = q.reshape(nwin, CPW, P, G, C_OUT).transpose(2, 0, 3, 1, 4)
        shard = np.ascontiguousarray(shard).reshape(P, nwin * 2 * CPW * C_OUT)
        relk = np.ascontiguousarray(
            ck['rel'].T.astype(np.float32)).astype(ml_dtypes.bfloat16)
        in_maps.append(dict(pts=shard, rel=relk, iota=iota_c))

    nc = _build_program(nchunk)
    res = _run_on_hw(nc, in_maps, trace=_trace)

    # ---------------- host assembly ----------------
    s_parts = []
    v_parts = []
    for k in range(NCORES):
        vals = np.asarray(res.results[k]['wout']).astype(np.float32)
        # [nwin, C, CPW*WIN] -> [nchunk, WIN, C]
        vals = vals.reshape(nwin, C_OUT, CPW, WIN).transpose(0, 2, 3, 1)
        vals = vals.reshape(nchunk, WIN, C_OUT) * scales[k][:, None, None]
        cseg = plan['cores'][k]['slot_seg']
        m = cseg >= 0
        s_parts.append(cseg[m])
        v_parts.append(vals[m])
    s_all = np.concatenate(s_parts)
    v_all = np.concatenate(v_parts)
    acc = np.zeros((NSEG, C_OUT), np.float32)
    if len(s_all):
        o2 = np.argsort(s_all, kind='stable')
        s2 = s_all[o2]
        v2 = v_all[o2]
        starts = np.r_[0, np.flatnonzero(np.diff(s2)) + 1]
        sums = np.add.reduceat(v2, starts, axis=0)
        useg = s2[starts]
        acc[useg] = sums / np.maximum(plan['counts'][useg], 1.0)[:, None]

    out = acc.reshape(NX[2], NX[0], NX[1], C_OUT).transpose(0, 3, 1, 2)
    out = out.reshape(1, NX[2] * C_OUT, NX[0], NX[1]).astype(np.float32)
    if _return_results:
        return out, res
    return out


# revision 6
# speedup vs baseline: 1.5113x; 1.2210x over previous
"""Trainium2 Bass kernel for nn_BaseViewTransform (BEVFusion bev_pool / segment-mean).

Pipeline (v2b — int8 stream, device pair-add, feats-stationary FWL matmul):
  Host (index plane only, derived from the 5 small input matrices):
    - compute per-point voxel/segment ids exactly as the reference
    - sort kept points by segment id; pair adjacent same-segment points into
      pseudo-points (odd tails padded with a zero slot)
    - shard pseudo-points across 8 cores; greedy-pack into 128-pseudo chunks
      with <= WIN=12 distinct segments per chunk
    - per-chunk symmetric int8 quantization (scale = absmax/127)
  Device (single SPMD program), per 42-chunk window:
    - stream int8 points (evens block + odds block, contiguous)
    - DVE + GPSIMD split: pair-add int8+int8 -> bf16 (exact: |q1+q2| <= 254)
    - DVE: one-hot [128, 42, 12] via is_equal on iota/rel
    - 42 matmuls: pair-sums stationary (lhsT padded to 128 cols so FWL
      triggers), one-hot moving (12 cols); PSUM [128, 42*12]
    - ACT: PSUM [0:80] -> SBUF bf16 copy; DMA out
  Host: scale chunk sums by the chunk scale, reduce per segment, divide by
  counts, scatter into the [1, 80, 360, 360] grid.
"""

import numpy as np
import ml_dtypes

# ---------------- problem constants (hardcoded per task rules) ----------------
IMAGE_SIZE = (256, 704)
FEATURE_SIZE = (32, 88)
XBOUND = (-54.0, 54.0, 0.3)
YBOUND = (-54.0, 54.0, 0.3)
ZBOUND = (-10.0, 10.0, 20.0)
DBOUND = (1.0, 60.0, 0.5)
C_OUT = 80
NX = (360, 360, 1)
NSEG = NX[2] * NX[0] * NX[1]  # 129600
DX = np.array([XBOUND[2], YBOUND[2], ZBOUND[2]], np.float32)
BX = np.array([XBOUND[0] + XBOUND[2] / 2.0,
               YBOUND[0] + YBOUND[2] / 2.0,
               ZBOUND[0] + ZBOUND[2] / 2.0], np.float32)

NCORES = 8
P = 128          # pseudo-points per chunk (= matmul contraction dim)
G = 2            # original points per pseudo-point (device pair-add)
WIN = 12         # max distinct segments per chunk (= one-hot width)
CPW = 42         # chunks per window (42*12 = 504 <= 512 fp32 PSUM bank)
FW = G * C_OUT   # feature elements per chunk per partition (160)
NS = 42          # chunks of each window pair-added on DVE (rest on GPSIMD)
PAD = 0          # lhsT column padding (0 = no FWL, saves LDW read bandwidth)


def _frustum():
    iH, iW = IMAGE_SIZE
    fH, fW = FEATURE_SIZE
    ds = np.arange(DBOUND[0], DBOUND[1], DBOUND[2], dtype=np.float32)
    xs = np.linspace(0.0, iW - 1.0, fW, dtype=np.float32)
    ys = np.linspace(0.0, iH - 1.0, fH, dtype=np.float32)
    return np.stack(np.broadcast_arrays(
        xs[None, None, :], ys[None, :, None], ds[:, None, None]), -1
    ).astype(np.float32)  # [D, fH, fW, 3]


def _segments(camera_intrinsics, camera2lidar, img_aug_matrix, lidar_aug_matrix):
    """Replicates reference get_geometry + voxelization in numpy float32.
    Returns (seg[Np] int64, kept[Np] bool)."""
    intr = np.asarray(camera_intrinsics, np.float32)
    c2l = np.asarray(camera2lidar, np.float32)
    img_aug = np.asarray(img_aug_matrix, np.float32)
    lidar_aug = np.asarray(lidar_aug_matrix, np.float32)

    intrins = intr[..., :3, :3]
    post_rots = img_aug[..., :3, :3]
    post_trans = img_aug[..., :3, 3]
    rots = c2l[..., :3, :3]
    trans = c2l[..., :3, 3]
    er = lidar_aug[..., :3, :3]
    et = lidar_aug[..., :3, 3]

    f = _frustum()
    pts = f[None, None] - post_trans[:, :, None, None, None, :]
    ipr = np.linalg.inv(post_rots.astype(np.float64)).astype(np.float32)
    pts = np.einsum('bnij,bndhwj->bndhwi', ipr, pts).astype(np.float32)
    pts = np.concatenate([pts[..., :2] * pts[..., 2:3], pts[..., 2:3]], -1)
    iintr = np.linalg.inv(intrins.astype(np.float64)).astype(np.float32)
    comb = np.einsum('bnij,bnjk->bnik', rots, iintr).astype(np.float32)
    pts = (np.einsum('bnij,bndhwj->bndhwi', comb, pts)
           + trans[:, :, None, None, None, :]).astype(np.float32)
    pts = (np.einsum('bij,bndhwj->bndhwi', er, pts)
           + et[:, None, None, None, None, :]).astype(np.float32)

    Np_ = pts.size // 3
    geom = ((pts - (BX - DX / 2.0)) / DX).astype(np.int32).reshape(Np_, 3)
    kept = ((geom[:, 0] >= 0) & (geom[:, 0] < NX[0])
            & (geom[:, 1] >= 0) & (geom[:, 1] < NX[1])
            & (geom[:, 2] >= 0) & (geom[:, 2] < NX[2]))
    seg = (geom[:, 2].astype(np.int64) * (NX[0] * NX[1])
           + geom[:, 0].astype(np.int64) * NX[1]
           + geom[:, 1].astype(np.int64))
    return seg, kept


def _plan(seg, kept):
    """Sort kept points, pair into pseudo-points, shard, greedy-chunk.

    Returns per-core rows [nchunk,P,G] (point row ids, -1 pad), rel [nchunk,P]
    (one-hot column, -1 pad), slot_seg [nchunk,WIN] (segment per slot, -1
    unused), plus global counts.
    """
    kidx = np.nonzero(kept)[0].astype(np.int64)
    segk = seg[kidx]
    order = np.argsort(segk, kind='stable')
    rows_sorted = kidx[order]
    seg_sorted = segk[order]
    counts = np.bincount(seg_sorted, minlength=NSEG).astype(np.float32)

    rs = np.flatnonzero(np.r_[True, np.diff(seg_sorted) != 0]).astype(np.int64)
    rlen = np.diff(np.r_[rs, len(seg_sorted)]).astype(np.int64)
    run_seg = seg_sorted[rs]
    plen = (rlen + G - 1) // G
    pcum = np.concatenate([[0], np.cumsum(plen)])
    npseudo = int(pcum[-1])
    bounds = [int(round(npseudo * k / NCORES)) for k in range(NCORES + 1)]

    cores_pieces = []
    ri = 0
    for k in range(NCORES):
        lo, hi = bounds[k], bounds[k + 1]
        while ri + 1 < len(pcum) and pcum[ri + 1] <= lo:
            ri += 1
        # pieces: (run_id, take, run_poff, slot, chunk)
        pieces = []
        q = 0   # local pseudo cursor incl. padding
        d = 0   # distinct segs in current chunk
        p = lo
        rj = ri
        while p < hi:
            run_end = pcum[rj + 1]
            rem = int(min(run_end, hi) - p)
            poff = int(p - pcum[rj])
            while rem:
                cpos = q % P
                if cpos == 0:
                    d = 0
                if d == WIN:  # chunk out of slots: pad to chunk end
                    q += P - cpos
                    d = 0
                    cpos = 0
                slot = d
                d += 1
                take = min(rem, P - cpos)
                pieces.append((rj, take, poff, slot, q // P))
                q += take
                rem -= take
                poff += take
            p = int(min(run_end, hi))
            if p >= run_end:
                rj += 1
        cores_pieces.append((pieces, q))

    nchunk = max((q + P - 1) // P for _, q in cores_pieces)
    nchunk = ((nchunk + CPW - 1) // CPW) * CPW

    out = []
    for k, (pieces, q) in enumerate(cores_pieces):
        Q = nchunk * P
        run_f = np.full(Q, -1, np.int64)
        poff_f = np.zeros(Q, np.int64)
        rel_f = np.full(Q, -1, np.int32)
        slot_seg = np.full((nchunk, WIN), -1, np.int64)
        pos = 0
        for (rj, take, poff, slot, c) in pieces:
            cpos = pos % P
            if pos // P != c:  # padding was inserted before this piece
                pos = c * P
            run_f[pos:pos + take] = rj
            poff_f[pos:pos + take] = np.arange(poff, poff + take)
            rel_f[pos:pos + take] = slot
            slot_seg[c, slot] = run_seg[rj]
            pos += take
        valid = run_f >= 0
        rows = np.full((Q, G), -1, np.int64)
        base = rs[run_f[valid]] + poff_f[valid] * G
        end = rs[run_f[valid]] + rlen[run_f[valid]]
        for j in range(G):
            idx = base + j
            ok = idx < end
            rr = np.full(int(valid.sum()), -1, np.int64)
            rr[ok] = rows_sorted[idx[ok]]
            rows[valid, j] = rr
        out.append(dict(rows=rows.reshape(nchunk, P, G),
                        rel=rel_f.reshape(nchunk, P).astype(np.int32),
                        slot_seg=slot_seg))
    return dict(nchunk=nchunk, cores=out, counts=counts)


# ---------------- device program ----------------
_COMPILED = {}


def _build_program(nchunk):
    import concourse.tile as tile
    from concourse import bacc, mybir
    import concourse.bass as bass

    key = (nchunk, G, WIN)
    if key in _COMPILED:
        return _COMPILED[key]

    nwin = nchunk // CPW
    bf = mybir.dt.bfloat16
    i8 = mybir.dt.int8
    WB = CPW * C_OUT            # feature elems per window block (3360)
    nc = bacc.Bacc("TRN2", target_bir_lowering=False, debug=False,
                   enable_asserts=False, num_devices=NCORES)
    pts = nc.dram_tensor("pts", [P, nwin * 2 * WB], i8,
                         kind="ExternalInput").ap()
    rel = nc.dram_tensor("rel", [P, nchunk], bf, kind="ExternalInput").ap()
    iota = nc.dram_tensor("iota", [P, WIN], bf, kind="ExternalInput").ap()
    wout = nc.dram_tensor("wout", [nwin, C_OUT, CPW * WIN], bf,
                          kind="ExternalOutput").ap()

    with tile.TileContext(nc) as tc:
        with tc.tile_pool(name="const", bufs=1) as constp, \
             tc.tile_pool(name="feat", bufs=3) as featp, \
             tc.tile_pool(name="pair", bufs=3) as pairp, \
             tc.tile_pool(name="oh", bufs=3) as ohp, \
             tc.tile_pool(name="stage", bufs=3) as stagep, \
             tc.tile_pool(name="psum", bufs=4, space="PSUM") as psump:
            rel_t = constp.tile([P, nchunk], bf)
            nc.sync.dma_start(out=rel_t[:], in_=rel[:])
            iota_t = constp.tile([P, WIN], bf)
            nc.sync.dma_start(out=iota_t[:], in_=iota[:])

            for w in range(nwin):
                f_t = featp.tile([P, 2 * WB], i8)
                eng = nc.sync if w % 2 == 0 else nc.scalar
                eng.dma_start(out=f_t[:], in_=pts[:, w * 2 * WB:(w + 1) * 2 * WB])

                # pair-add int8+int8 -> bf16, split DVE / GPSIMD
                pr = pairp.tile([P, WB + PAD], bf)
                sp = NS * C_OUT
                nc.vector.tensor_tensor(
                    out=pr[:, 0:sp], in0=f_t[:, 0:sp],
                    in1=f_t[:, WB:WB + sp], op=mybir.AluOpType.add)
                if sp < WB:
                    nc.gpsimd.tensor_tensor(
                        out=pr[:, sp:WB], in0=f_t[:, sp:WB],
                        in1=f_t[:, WB + sp:2 * WB], op=mybir.AluOpType.add)

                # one-hot for the window via DVE compare
                oh = ohp.tile([P, CPW, WIN], bf)
                rsl = rel_t[:, w * CPW:(w + 1) * CPW]
                rel_b = bass.AP(rsl.tensor, rsl.offset,
                                list(rsl.ap) + [[0, WIN]])
                iap = iota_t[:]
                iota_b = bass.AP(iap.tensor, iap.offset,
                                 [iap.ap[0], [0, CPW], iap.ap[1]])
                nc.vector.tensor_tensor(out=oh[:], in0=iota_b, in1=rel_b,
                                        op=mybir.AluOpType.is_equal)

                ps = psump.tile([P, 512], mybir.dt.float32)
                for c in range(CPW):
                    lhsT = pr[:, c * C_OUT:c * C_OUT + C_OUT + PAD]
                    nc.tensor.matmul(
                        out=ps[0:C_OUT + PAD, c * WIN:(c + 1) * WIN],
                        lhsT=lhsT,
                        rhs=oh[:, c],
                        start=True,
                        stop=True,
                    )
                st = stagep.tile([C_OUT, CPW * WIN], bf)
                nc.scalar.copy(out=st[:], in_=ps[0:C_OUT, 0:CPW * WIN])
                nc.gpsimd.dma_start(out=wout[w], in_=st[:])

    nc.compile()
    _COMPILED[key] = nc
    return nc


def _run_on_hw(nc, in_maps, trace=False):
    from concourse.bass_utils import run_bass_kernel_spmd
    from concourse.bass_interp import get_hw_module

    if trace:
        try:
            import ntff_hook
            ntff_hook.install()
        except Exception:
            pass
    hw_m = get_hw_module(nc.m)
    old_m = nc.m
    nc.m = hw_m
    try:
        res = run_bass_kernel_spmd(
            nc, in_maps, core_ids=list(range(NCORES)), trace=trace,
        )
    finally:
        nc.m = old_m
    return res


def kernel(cam_feats, camera_intrinsics, camera2lidar, img_aug_matrix,
           lidar_aug_matrix, _trace=False, _return_results=False):
    cam = np.ascontiguousarray(np.asarray(cam_feats, np.float32))
    Npts = cam.size // C_OUT
    cam_f = cam.reshape(Npts, C_OUT)
    cam_aug = np.vstack([cam_f, np.zeros((1, C_OUT), np.float32)])

    seg, kept = _segments(camera_intrinsics, camera2lidar,
                          img_aug_matrix, lidar_aug_matrix)
    plan = _plan(seg, kept)
    nchunk = plan['nchunk']
    nwin = nchunk // CPW

    iota_c = np.broadcast_to(np.arange(WIN, dtype=np.float32),
                             (P, WIN)).astype(ml_dtypes.bfloat16)
    in_maps = []
    scales = []
    for k in range(NCORES):
        ck = plan['cores'][k]
        f = cam_aug[ck['rows'].reshape(-1)].reshape(nchunk, P, G, C_OUT)
        a = np.abs(f).reshape(nchunk, P * G * C_OUT).max(axis=1)
        s = np.where(a > 0, a / 127.0, 1.0).astype(np.float32)
        q = np.rint(f / s[:, None, None, None])
        q = np.clip(q, -127, 127).astype(np.int8)
        scales.append(s)
        # [nchunk,P,G,C] -> [P, nwin, G, CPW, C] (evens block, odds block)
        shard = q.reshape(nwin, CPW, P, G, C_OUT).transpose(2, 0, 3, 1, 4)
        shard = np.ascontiguousarray(shard).reshape(P, nwin * 2 * CPW * C_OUT)
        relk = np.ascontiguousarray(
            ck['rel'].T.astype(np.float32)).astype(ml_dtypes.bfloat16)
        in_maps.append(dict(pts=shard, rel=relk, iota=iota_c))

    nc = _build_program(nchunk)
    res = _run_on_hw(nc, in_maps, trace=_trace)

    # ---------------- host assembly ----------------
    s_parts = []
    v_parts = []
    for k in range(NCORES):
        vals = np.asarray(res.results[k]['wout']).astype(np.float32)
        # [nwin, C, CPW*WIN] -> [nchunk, WIN, C]
        vals = vals.reshape(nwin, C_OUT, CPW, WIN).transpose(0, 2, 3, 1)
        vals = vals.reshape(nchunk, WIN, C_OUT) * scales[k][:, None, None]
        cseg = plan['cores'][k]['slot_seg']
        m = cseg >= 0
        s_parts.append(cseg[m])
        v_parts.append(vals[m])
    s_all = np.concatenate(s_parts)
    v_all = np.concatenate(v_parts)
    acc = np.zeros((NSEG, C_OUT), np.float32)
    if len(s_all):
        o2 = np.argsort(s_all, kind='stable')
        s2 = s_all[o2]
        v2 = v_all[o2]
        starts = np.r_[0, np.flatnonzero(np.diff(s2)) + 1]
        sums = np.add.reduceat(v2, starts, axis=0)
        useg = s2[starts]
        acc[useg] = sums / np.maximum(plan['counts'][useg], 1.0)[:, None]

    out = acc.reshape(NX[2], NX[0], NX[1], C_OUT).transpose(0, 3, 1, 2)
    out = out.reshape(1, NX[2] * C_OUT, NX[0], NX[1]).astype(np.float32)
    if _return_results:
        return out, res
    return out
